# revision 1
# baseline (speedup 1.0000x reference)
"""Trainium2 Bass kernel for nn_EdgeFocusedGraphNetwork.

Math: the reference's edge tensor fe[b,i,j,:] stays rank-structured for the
whole computation -- every edge update is affine and the injected new_e is an
outer sum, so fe = X[b,i,:] + Y[b,j,:] + c[:] inductively. The softmax over the
source index i is shift-invariant, which cancels the Y and c components, and
the softmax weights / aggregation become independent of j. The whole network
therefore collapses exactly (in real arithmetic) to (l, h)-sized operations per
batch element. Additionally the X recurrence is expanded through the (linear)
attention projection, so P_t = X_t @ W_attn.T = sum_s fv_s @ G_{t,s} with
host-precomputed G matrices:

    fv_0 = feat @ W_inp.T + b_inp
    P_t  = sum_{s<=t} fv_s @ G_{t,s}
    xh_t = ((fv_t @ W_agg.T) * mask + b_agg)        (mask is per-token scalar)
    w    = softmax_i(P_t[i,h]);  s[h] = sum_i w[i,h] * xh_t[i,h]
    fv_{t+1} = xh_t @ Wuv1.T + (sigmoid(s) @ Wuv2.T + b_uv)
    out  = fv_3 @ W_oup.T + b_oup

Sharding: data-parallel over batch, one batch element per NeuronCore (b=8 ->
8 cores); weights (host-precombined in float64) replicated.

Device layout: feature dim on partitions (2 blocks of 128), tokens on the free
dim, so the softmax is a free-axis reduction. feat is transposed on-chip via
PE transposes (identity generated on-device); the final projection is emitted
token-on-partition so the output DMA is contiguous, with b_oup injected via a
K=1 ones-row matmul at the start of the PSUM group. Sigmoid is computed as
1/(1+exp(-s)) so every ACT instruction uses the exp/identity LUT set (single
table load). Softmax max-subtraction is skipped: |P| < 1 for this model's
weight/input scaling (verified), so exp is exact-safe.

Weights are host-packed into five device-layout segments, one contiguous DMA
each, issued on the sync engine in exact need order (HWDGE issue overhead is
~650ns per DMA and serializes, and the shared DMA path drains in arrival
order, so few big DMAs in need order beat many small or out-of-order ones).
"""

import sys

for _p in ("/opt/trn_rl_repo",):
    if _p not in sys.path:
        sys.path.insert(0, _p)

from contextlib import ExitStack

import numpy as np

import concourse.bass as bass
import concourse.tile as tile
from concourse import bacc, mybir, bass_utils
from concourse.masks import make_identity

F32 = mybir.dt.float32
L = 128          # tokens per graph
H = 256          # inner width
F = 512          # in/out feature width
NSTEP = 3
NCORES = 8
HH = H // 128    # 2 feature half-blocks
FH = F // 128    # 4 feature blocks

AF = mybir.ActivationFunctionType
ALU = mybir.AluOpType
AX = mybir.AxisListType

# packed segment column layouts (per 128-partition row, in f32 elements)
#   seg0: A_inp (FH*H) | b_inp (HH) | b_agg (HH) | b_uv (HH)
#   seg1a: A_agg | G1   seg1b: A_uv1 | A_uv2    (each HH*H = 512 cols)
#   seg2: G2 | G3 | G4 | G5
#   seg3: A_oup (HH*F = 1024 cols)
SEG0_COLS = FH * H + 3 * HH
SEG1_COLS = 2 * HH * H
SEG2_COLS = 4 * HH * H
SEG3_COLS = HH * F

_W_NAMES = [
    ("seg0", (128, SEG0_COLS)),
    ("seg1a", (128, SEG1_COLS)),
    ("seg1b", (128, SEG1_COLS)),
    ("seg2", (128, SEG2_COLS)),
    ("seg3", (128, SEG3_COLS)),
    ("b_oup_row", (1, F)),
]

_SEG1A_ORDER = ("A_agg", "G1")
_SEG1B_ORDER = ("A_uv1", "A_uv2")
_SEG2_ORDER = ("G2", "G3", "G4", "G5")

# G matrix used for fv_s's contribution to P_t, [t][s]
_G_SCHED = [["G1"], ["G3", "G2"], ["G5", "G4", "G2"]]


def _emit(tc, io):
    nc = tc.nc
    with ExitStack() as ctx:
        const = ctx.enter_context(tc.tile_pool(name="const", bufs=1))
        state = ctx.enter_context(tc.tile_pool(name="state", bufs=4))
        work = ctx.enter_context(tc.tile_pool(name="work", bufs=3))
        psA = ctx.enter_context(tc.tile_pool(name="psA", bufs=4, space="PSUM"))
        psO = ctx.enter_context(tc.tile_pool(name="psO", bufs=2, space="PSUM"))

        # ---- inputs / constants into SBUF ----
        feat_sb = const.tile([128, F], F32)
        nc.sync.dma_start(feat_sb[:], io["feat"])
        seg0 = const.tile([128, SEG0_COLS], F32)
        nc.sync.dma_start(seg0[:], io["seg0"])
        seg1a = const.tile([128, SEG1_COLS], F32)
        nc.sync.dma_start(seg1a[:], io["seg1a"])

        maskb = const.tile([128, L], F32)  # mask broadcast to all partitions
        m = io["mask"]
        nc.sync.dma_start(
            maskb[:],
            bass.AP(tensor=m.tensor, offset=m.offset, ap=[[0, 128]] + list(m.ap)),
        )

        seg1b = const.tile([128, SEG1_COLS], F32)
        nc.sync.dma_start(seg1b[:], io["seg1b"])
        seg2 = const.tile([128, SEG2_COLS], F32)
        nc.sync.dma_start(seg2[:], io["seg2"])
        seg3 = const.tile([128, SEG3_COLS], F32)
        nc.sync.dma_start(seg3[:], io["seg3"])
        b_oup_sb = const.tile([1, F], F32)
        nc.sync.dma_start(b_oup_sb[:], io["b_oup_row"])

        ident = const.tile([128, 128], F32)
        make_identity(nc, ident[:])
        ones_row = const.tile([1, 128], F32)
        nc.vector.memset(ones_row[:], 1.0)

        # weight/bias slice helpers into the packed segments
        def a_inp(k, c):
            o = k * H + c * 128
            return seg0[:, o:o + 128]

        _b_off = {"b_inp": FH * H, "b_agg": FH * H + HH, "b_uv": FH * H + 2 * HH}

        def bias(name, c):
            o = _b_off[name] + c
            return seg0[:, o:o + 1]

        _w_seg = {}
        for i, nm in enumerate(_SEG1A_ORDER):
            _w_seg[nm] = (seg1a, i * HH * H)
        for i, nm in enumerate(_SEG1B_ORDER):
            _w_seg[nm] = (seg1b, i * HH * H)
        for i, nm in enumerate(_SEG2_ORDER):
            _w_seg[nm] = (seg2, i * HH * H)

        def wmat(name, k, c):
            t, base = _w_seg[name]
            o = base + k * H + c * 128
            return t[:, o:o + 128]

        def a_oup(k):
            return seg3[:, k * F:(k + 1) * F]

        # ---- featT[p, k, l] = feat[l, 128k + p] via PE transposes ----
        featT = const.tile([128, FH, 128], F32)
        for k in range(FH):
            pst = psA.tile([128, 128], F32, tag="ps", name="pst")
            nc.tensor.transpose(pst[:], feat_sb[:, k * 128:(k + 1) * 128], ident[:])
            nc.vector.tensor_copy(featT[:, k, :], pst[:])

        # ---- fv_0 = feat @ W_inp.T + b_inp (feature-on-partition layout) ----
        fvs = []
        fv0 = state.tile([128, HH, 128], F32, tag="fvT", name="fv0")
        for c in range(HH):
            psf = psA.tile([128, 128], F32, tag="ps", name="psf")
            for k in range(FH):
                nc.tensor.matmul(
                    psf[:], a_inp(k, c), featT[:, k, :],
                    start=(k == 0), stop=(k == FH - 1),
                )
            nc.scalar.activation(
                fv0[:, c, :], psf[:], AF.Identity, bias=bias("b_inp", c)
            )
        fvs.append(fv0)

        # P_0 accumulators (no old terms for step 0)
        psP = [psA.tile([128, 128], F32, tag="ps", name="psP") for _ in range(HH)]
        started = [False, False]

        for t_step in range(NSTEP):
            fv_t = fvs[t_step]
            gnames = _G_SCHED[t_step]

            # ---- z = fv_t @ W_agg.T (masked + biased below) ----
            psZ = []
            for c in range(HH):
                p = psA.tile([128, 128], F32, tag="psz", name="psZ", bufs=2)
                psZ.append(p)
                for k in range(HH):
                    nc.tensor.matmul(
                        p[:], wmat("A_agg", k, c), fv_t[:, k, :],
                        start=(k == 0), stop=(k == HH - 1),
                    )

            # ---- P_t final term (needs fv_t) ----
            for c in range(HH):
                for k in range(HH):
                    nc.tensor.matmul(
                        psP[c][:], wmat(gnames[t_step], k, c), fv_t[:, k, :],
                        start=(not started[c] and k == 0), stop=(k == HH - 1),
                    )
                started[c] = True

            # ---- xh = z * mask + b_agg ----
            xh = work.tile([128, HH, 128], F32, tag="xh", name="xh", bufs=2)
            xz = work.tile([128, HH, 128], F32, tag="xz", name="xz")
            for c in range(HH):
                nc.vector.tensor_tensor(xz[:, c, :], psZ[c][:], maskb[:], op=ALU.mult)
                nc.scalar.activation(
                    xh[:, c, :], xz[:, c, :], AF.Identity, bias=bias("b_agg", c)
                )

            # ---- softmax over tokens (|P| < 1: no max subtraction),
            #      s = <w, xh>, sig = 1/(1+exp(-s)) ----
            e = work.tile([128, HH, 128], F32, tag="e", name="e")
            for c in range(HH):
                nc.scalar.activation(e[:, c, :], psP[c][:], AF.Exp)
            sen = work.tile([128, HH], F32, tag="sen", name="sen")
            nc.vector.reduce_sum(sen[:], e[:], axis=AX.X, negate=True)
            recn = work.tile([128, HH], F32, tag="recn", name="recn")
            nc.vector.reciprocal(recn[:], sen[:])           # -1/sum(e)
            prod = work.tile([128, HH, 128], F32, tag="prod", name="prod")
            nc.vector.tensor_mul(prod[:], e[:], xh[:])
            num = work.tile([128, HH], F32, tag="num", name="num")
            nc.vector.reduce_sum(num[:], prod[:], axis=AX.X)
            es = work.tile([128, HH], F32, tag="es", name="es")
            for c in range(HH):                             # exp(-num/sum(e))
                nc.scalar.activation(
                    es[:, c:c + 1], num[:, c:c + 1], AF.Exp,
                    scale=recn[:, c:c + 1],
                )
            es1 = work.tile([128, HH], F32, tag="es1", name="es1")
            nc.vector.tensor_scalar_add(es1[:], es[:], 1.0)
            sig = work.tile([128, HH], F32, tag="sig", name="sig")
            nc.vector.reciprocal(sig[:], es1[:])

            # ---- fv_{t+1} matmuls (only need xh) run before sig-dependent work
            psf2s = []
            for c in range(HH):
                psf2 = psA.tile([128, 128], F32, tag="ps", name="psf2")
                psf2s.append(psf2)
                for k in range(HH):
                    nc.tensor.matmul(
                        psf2[:], wmat("A_uv1", k, c), xh[:, k, :],
                        start=(k == 0), stop=(k == HH - 1),
                    )

            # ---- next step's P old terms (all source fvs already exist) ----
            if t_step < NSTEP - 1:
                gnext = _G_SCHED[t_step + 1]
                psPn = [
                    psA.tile([128, 128], F32, tag="ps", name="psPn")
                    for _ in range(HH)
                ]
                startedn = [False, False]
                for c in range(HH):
                    for s in range(t_step + 1):
                        for k in range(HH):
                            nc.tensor.matmul(
                                psPn[c][:], wmat(gnext[s], k, c), fvs[s][:, k, :],
                                start=(s == 0 and k == 0), stop=False,
                            )
                    startedn[c] = True

            # ---- rank-1 term vb = A_uv2-matvec(sig) + b_uv ----
            vb = work.tile([128, HH], F32, tag="vb", name="vb")
            for c in range(HH):
                psv = psA.tile([128, 1], F32, tag="psz", name="psv", bufs=2)
                for k in range(HH):
                    nc.tensor.matmul(
                        psv[:], wmat("A_uv2", k, c), sig[:, k:k + 1],
                        start=(k == 0), stop=(k == HH - 1),
                    )
                nc.vector.tensor_add(vb[:, c:c + 1], psv[:], bias("b_uv", c))

            # ---- fv_{t+1} = xh @ Wuv1.T + vb ----
            fvn = state.tile([128, HH, 128], F32, tag="fvT", name="fvn")
            for c in range(HH):
                nc.scalar.activation(
                    fvn[:, c, :], psf2s[c][:], AF.Identity, bias=vb[:, c:c + 1]
                )
            fvs.append(fvn)
            if t_step < NSTEP - 1:
                psP = psPn
                started = startedn

        # ---- out = fv_3 @ W_oup.T + b_oup (token-on-partition orientation),
        #      two free-halves so the first output DMA overlaps the second
        #      half's matmuls ----
        fv3 = fvs[NSTEP]
        HF = F // 2
        for h2 in range(2):
            off = h2 * HF
            pso = psO.tile([128, HF], F32, tag="pso", name="pso")
            nc.tensor.matmul(
                pso[:], ones_row[:], b_oup_sb[:, off:off + HF],
                start=True, stop=False,
            )
            for k in range(HH):
                nc.tensor.matmul(
                    pso[:], fv3[:, k, :], seg3[:, k * F + off:k * F + off + HF],
                    start=False, stop=(k == HH - 1),
                )
            out_sb = work.tile([128, HF], F32, tag="out", name="out_sb", bufs=2)
            nc.vector.tensor_copy(out_sb[:], pso[:])
            nc.sync.dma_start(io["out"][:, off:off + HF], out_sb[:])


_NC_CACHE = []


def _build():
    if _NC_CACHE:
        return _NC_CACHE[0]
    nc = bacc.Bacc("TRN2", target_bir_lowering=False, debug=False,
                   num_devices=NCORES)
    io = {}
    io["feat"] = nc.dram_tensor("feat", (L, F), F32, kind="ExternalInput").ap()
    io["mask"] = nc.dram_tensor("mask", (L,), F32, kind="ExternalInput").ap()
    for name, shape in _W_NAMES:
        io[name] = nc.dram_tensor(name, shape, F32, kind="ExternalInput").ap()
    io["out"] = nc.dram_tensor("out", (L, F), F32, kind="ExternalOutput").ap()
    with tile.TileContext(nc) as tc:
        _emit(tc, io)
    nc.compile()
    _NC_CACHE.append(nc)
    return nc


def _dev_mat(w):
    """(K, M) in-first weight -> device layout (128, K/128 * M)."""
    K, M = w.shape
    return w.reshape(K // 128, 128, M).transpose(1, 0, 2).reshape(128, -1)


def _prep_weights(inputs):
    """Host-side weight precombination (float64) + device-layout packing."""
    g = {k: np.asarray(v, np.float64) for k, v in inputs.items()}
    h = H
    Wfe1T = g["W_fe"][:, :h].T           # (h, h)
    U1 = g["W_ue"][:, :h].T
    U2 = g["W_ue"][:, h:].T
    M1 = Wfe1T @ U1
    M0 = M1 + Wfe1T @ U2
    A = g["W_attn"].T
    mats = {
        "A_agg": g["W_agg"].T,
        "G1": M0 @ A,
        "G2": M1 @ A,
        "G3": M0 @ U2 @ A,
        "G4": M1 @ U2 @ A,
        "G5": M0 @ U2 @ U2 @ A,
        "A_uv1": g["W_uv"][:, :h].T,
        "A_uv2": g["W_uv"][:, h:].T,
    }
    seg0 = np.concatenate(
        [_dev_mat(g["W_inp"].T)]
        + [g[b].reshape(HH, 128).T for b in ("b_inp", "b_agg", "b_uv")],
        axis=1,
    )
    seg1a = np.concatenate([_dev_mat(mats[nm]) for nm in _SEG1A_ORDER], axis=1)
    seg1b = np.concatenate([_dev_mat(mats[nm]) for nm in _SEG1B_ORDER], axis=1)
    seg2 = np.concatenate([_dev_mat(mats[nm]) for nm in _SEG2_ORDER], axis=1)
    seg3 = _dev_mat(g["W_oup"].T)
    w = {
        "seg0": seg0, "seg1a": seg1a, "seg1b": seg1b, "seg2": seg2, "seg3": seg3,
        "b_oup_row": g["b_oup"][None, :],
    }
    return {k: np.ascontiguousarray(v, dtype=np.float32) for k, v in w.items()}


def kernel(**inputs) -> np.ndarray:
    nc = _build()
    w = _prep_weights(inputs)
    feat = np.ascontiguousarray(np.asarray(inputs["feat"], np.float32))
    mask = np.ascontiguousarray(np.asarray(inputs["mask"], np.float32))
    assert feat.shape == (NCORES, L, F), feat.shape

    in_maps = []
    for c in range(NCORES):
        im = {"feat": feat[c], "mask": mask[c]}
        im.update(w)
        in_maps.append(im)

    res = bass_utils.run_bass_kernel_spmd(nc, in_maps, core_ids=list(range(NCORES)))
    out = np.stack([res.results[c]["out"] for c in range(NCORES)], axis=0)
    return out.astype(np.float32)


if __name__ == "__main__":
    rng = np.random.default_rng(0)
    demo = {
        "feat": rng.standard_normal((NCORES, L, F)).astype(np.float32),
        "mask": np.ones((NCORES, L), np.float32),
    }
    for nm, shape in [("W_inp", (H, F)), ("b_inp", (H,)), ("W_oup", (F, H)),
                      ("b_oup", (F,)), ("W_fe", (H, 2 * H)), ("b_fe", (H,)),
                      ("W_ue", (H, 2 * H)), ("b_ue", (H,)), ("W_agg", (H, H)),
                      ("b_agg", (H,)), ("W_uv", (H, 2 * H)), ("b_uv", (H,)),
                      ("W_attn", (H, H)), ("b_attn", (H,))]:
        demo[nm] = (rng.standard_normal(shape) * 0.05).astype(np.float32)
    y = kernel(**demo)
    print("kernel output:", y.shape, y.dtype)



# revision 4
# speedup vs baseline: 1.3188x; 1.3188x over previous
"""Trainium2 Bass kernel for nn_EdgeFocusedGraphNetwork.

Math: the reference's edge tensor fe[b,i,j,:] stays rank-structured for the
whole computation -- every edge update is affine and the injected new_e is an
outer sum, so fe = X[b,i,:] + Y[b,j,:] + c[:] inductively. The softmax over the
source index i is shift-invariant, which cancels the Y and c components, and
the softmax weights / aggregation become independent of j. The whole network
therefore collapses exactly (in real arithmetic) to (l, h)-sized operations per
batch element. Additionally the X recurrence is expanded through the (linear)
attention projection, so P_t = X_t @ W_attn.T = sum_s fv_s @ G_{t,s} with
host-precomputed G matrices:

    fv_0 = feat @ W_inp.T + b_inp
    P_t  = sum_{s<=t} fv_s @ G_{t,s}
    xh_t = ((fv_t @ W_agg.T) * mask + b_agg)        (mask is per-token scalar)
    w    = softmax_i(P_t[i,h]);  s[h] = sum_i w[i,h] * xh_t[i,h]
    fv_{t+1} = xh_t @ Wuv1.T + (sigmoid(s) @ Wuv2.T + b_uv)
    out  = fv_3 @ W_oup.T + b_oup

Sharding: data-parallel over batch, one batch element per NeuronCore (b=8 ->
8 cores); weights (host-precombined in float64) replicated.

Device layout: feature dim on partitions (2 blocks of 128), tokens on the free
dim, so the softmax is a free-axis reduction. All matmul operands are bf16
(PSUM accumulation stays fp32): 1 cycle/row on the PE vs 4 for fp32, and half
the weight-DMA bytes. feat is cast fp32->bf16 by a gpsimd (SWDGE) DMA, then
transposed on-chip via PE transposes with a bf16 identity. Softmax
max-subtraction is skipped: |P| < 1 for this model's weight/input scaling.

Chain fusions: exp emits its own denominator via activation accum_out;
numerator via scalar_tensor_tensor(e, 0.5, xh) with accum_out (the 0.5 feeds
the tanh half-angle form); sigmoid(s) = 0.5*tanh(s/2)+0.5 (tanh shares the
exp LUT set, so no table reload) with the affine folded host-side into
A_uv2h = A_uv2/2 and c_uv = b_uv + A_uv2.T-colsum/2. The final projection is
emitted token-on-partition so the output DMA is contiguous, with b_oup
injected via a K=1 ones-row matmul.

DMA: weight segments are packed bf16 host-side, one contiguous DMA each,
issued in need order round-robin across the SP/Activation/DVE HWDGE lanes
(the ~650ns per-DMA issue serializes per engine, and HWDGE itself serializes
at ~625ns per DMA) while feat/mask go through the gpsimd SWDGE lane.
"""

import sys

for _p in ("/opt/trn_rl_repo",):
    if _p not in sys.path:
        sys.path.insert(0, _p)

from contextlib import ExitStack

import numpy as np
import ml_dtypes

import concourse.bass as bass
import concourse.tile as tile
from concourse import bacc, mybir, bass_utils
from concourse.masks import make_identity

F32 = mybir.dt.float32
BF16 = mybir.dt.bfloat16
L = 128          # tokens per graph
H = 256          # inner width
F = 512          # in/out feature width
NSTEP = 3
NCORES = 8
HH = H // 128    # 2 feature half-blocks
FH = F // 128    # 4 feature blocks

AF = mybir.ActivationFunctionType
ALU = mybir.AluOpType
AX = mybir.AxisListType

SEG0_COLS = FH * H          # A_inp
SEGP_COLS = 2 * HH * H      # a pair of HxH matrices
SEG3_COLS = HH * F          # A_oup
NCONST = 3 * HH             # b_inp | b_agg | c_uv columns (fp32)

_W_NAMES = [
    ("seg0", (128, SEG0_COLS), BF16),
    ("seg1a", (128, SEGP_COLS), BF16),
    ("seg1b", (128, SEGP_COLS), BF16),
    ("seg2a", (128, SEGP_COLS), BF16),
    ("seg2b", (128, SEGP_COLS), BF16),
    ("seg3", (128, SEG3_COLS), BF16),
    ("b_oup_row", (1, F), BF16),
    ("consts", (128, NCONST), F32),
]

_SEG_ORDER = {
    "seg1a": ("A_agg", "G1"),
    "seg1b": ("A_uv1", "A_uv2h"),
    "seg2a": ("G2", "G3"),
    "seg2b": ("G4", "G5"),
}

# G matrix used for fv_s's contribution to P_t, [t][s]
_G_SCHED = [["G1"], ["G3", "G2"], ["G5", "G4", "G2"]]


def _emit(tc, io):
    nc = tc.nc
    with ExitStack() as ctx:
        const = ctx.enter_context(tc.tile_pool(name="const", bufs=1))
        state = ctx.enter_context(tc.tile_pool(name="state", bufs=4))
        work = ctx.enter_context(tc.tile_pool(name="work", bufs=3))
        psA = ctx.enter_context(tc.tile_pool(name="psA", bufs=4, space="PSUM"))
        psO = ctx.enter_context(tc.tile_pool(name="psO", bufs=2, space="PSUM"))

        # ---- input / weight DMAs, spread across issue lanes in need order ----
        # gpsimd (SWDGE) lane: feat with fp32->bf16 cast, then mask broadcast
        feat_sb = const.tile([128, F], BF16)
        nc.gpsimd.dma_start(feat_sb[:], io["feat"])

        # HWDGE lanes (only SP + Activation have HWDGE), interleaved in need
        # order: SP gets seg0/seg1b/seg2b (+ output later), Act the rest.
        seg0 = const.tile([128, SEG0_COLS], BF16)
        nc.sync.dma_start(seg0[:], io["seg0"])
        consts = const.tile([128, NCONST], F32)
        nc.scalar.dma_start(consts[:], io["consts"])
        seg1b = const.tile([128, SEGP_COLS], BF16)
        nc.sync.dma_start(seg1b[:], io["seg1b"])
        seg1a = const.tile([128, SEGP_COLS], BF16)
        nc.scalar.dma_start(seg1a[:], io["seg1a"])

        # identity for PE transposes (gpsimd compute, between its DMAs)
        ident = const.tile([128, 128], BF16)
        make_identity(nc, ident[:])

        maskb = const.tile([128, L], F32)  # mask broadcast to all partitions
        m = io["mask"]
        nc.gpsimd.dma_start(
            maskb[:],
            bass.AP(tensor=m.tensor, offset=m.offset, ap=[[0, 128]] + list(m.ap)),
        )

        seg2b = const.tile([128, SEGP_COLS], BF16)
        nc.sync.dma_start(seg2b[:], io["seg2b"])
        seg2a = const.tile([128, SEGP_COLS], BF16)
        nc.scalar.dma_start(seg2a[:], io["seg2a"])
        seg3 = const.tile([128, SEG3_COLS], BF16)
        nc.scalar.dma_start(seg3[:], io["seg3"])
        b_oup_sb = const.tile([1, F], BF16)
        nc.gpsimd.dma_start(b_oup_sb[:], io["b_oup_row"])

        ones_row = const.tile([1, 128], BF16)
        nc.vector.memset(ones_row[:], 1.0)

        # weight/bias slice helpers into the packed segments
        def a_inp(k, c):
            o = k * H + c * 128
            return seg0[:, o:o + 128]

        _b_off = {"b_inp": 0, "b_agg": HH, "c_uv": 2 * HH}

        def bias(name, c):
            o = _b_off[name] + c
            return consts[:, o:o + 1]

        _w_seg = {}
        for segt, names in (
            (seg1a, _SEG_ORDER["seg1a"]),
            (seg1b, _SEG_ORDER["seg1b"]),
            (seg2a, _SEG_ORDER["seg2a"]),
            (seg2b, _SEG_ORDER["seg2b"]),
        ):
            for i, nm in enumerate(names):
                _w_seg[nm] = (segt, i * HH * H)

        def wmat(name, k, c):
            t, base = _w_seg[name]
            o = base + k * H + c * 128
            return t[:, o:o + 128]

        # ---- featT[p, k, l] = feat[l, 128k + p] via PE transposes (bf16) ----
        featT = const.tile([128, FH, 128], BF16)
        for k in range(FH):
            pst = psO.tile([128, 128], BF16, tag="pso", name="pst")
            nc.tensor.transpose(pst[:], feat_sb[:, k * 128:(k + 1) * 128], ident[:])
            nc.vector.tensor_copy(featT[:, k, :], pst[:])

        # ---- fv_0 = feat @ W_inp.T + b_inp (feature-on-partition layout) ----
        fvs = []
        fv0 = state.tile([128, HH, 128], BF16, tag="fvT", name="fv0")
        for c in range(HH):
            psf = psA.tile([128, 128], F32, tag="ps", name="psf")
            for k in range(FH):
                nc.tensor.matmul(
                    psf[:], a_inp(k, c), featT[:, k, :],
                    start=(k == 0), stop=(k == FH - 1),
                )
            nc.scalar.activation(
                fv0[:, c, :], psf[:], AF.Identity, bias=bias("b_inp", c)
            )
        fvs.append(fv0)

        # P_0 accumulators (no old terms for step 0)
        psP = [psA.tile([128, 128], F32, tag="ps", name="psP") for _ in range(HH)]
        started = [False, False]

        for t_step in range(NSTEP):
            fv_t = fvs[t_step]
            gnames = _G_SCHED[t_step]

            # ---- P_t final term first: it heads the softmax chain ----
            for c in range(HH):
                for k in range(HH):
                    nc.tensor.matmul(
                        psP[c][:], wmat(gnames[t_step], k, c), fv_t[:, k, :],
                        start=(not started[c] and k == 0), stop=(k == HH - 1),
                    )
                started[c] = True

            # ---- z = fv_t @ W_agg.T (masked + biased below) ----
            psZ = []
            for c in range(HH):
                p = psA.tile([128, 128], F32, tag="psz", name="psZ", bufs=2)
                psZ.append(p)
                for k in range(HH):
                    nc.tensor.matmul(
                        p[:], wmat("A_agg", k, c), fv_t[:, k, :],
                        start=(k == 0), stop=(k == HH - 1),
                    )

            # ---- next step's P old terms (no new dependencies: keep PE fed) ----
            if t_step < NSTEP - 1:
                gnext = _G_SCHED[t_step + 1]
                psPn = [
                    psA.tile([128, 128], F32, tag="ps", name="psPn")
                    for _ in range(HH)
                ]
                for c in range(HH):
                    for s in range(t_step + 1):
                        for k in range(HH):
                            nc.tensor.matmul(
                                psPn[c][:], wmat(gnext[s], k, c), fvs[s][:, k, :],
                                start=(s == 0 and k == 0), stop=False,
                            )

            # ---- e = exp(P) with fused denominator sen = sum_i e ----
            e = work.tile([128, HH, 128], BF16, tag="e", name="e")
            sen = work.tile([128, HH], F32, tag="sen", name="sen")
            for c in range(HH):
                nc.scalar.activation(
                    e[:, c, :], psP[c][:], AF.Exp, accum_out=sen[:, c:c + 1]
                )

            # ---- xh = z * mask + b_agg ----
            xz = work.tile([128, HH, 128], F32, tag="xz", name="xz")
            xh = work.tile([128, HH, 128], BF16, tag="xh", name="xh", bufs=2)
            for c in range(HH):
                nc.vector.tensor_tensor(xz[:, c, :], psZ[c][:], maskb[:], op=ALU.mult)
            recn = work.tile([128, HH], F32, tag="recn", name="recn")
            nc.vector.reciprocal(recn[:], sen[:])           # 1/sum(e)
            for c in range(HH):
                nc.scalar.activation(
                    xh[:, c, :], xz[:, c, :], AF.Identity, bias=bias("b_agg", c)
                )

            # ---- numerator: num2[h] = sum_i e*xh/2 (fused via accum_out) ----
            prod = work.tile([128, HH, 128], BF16, tag="prod", name="prod")
            num2 = work.tile([128, HH], F32, tag="num2", name="num2")
            for c in range(HH):
                nc.vector.scalar_tensor_tensor(
                    prod[:, c, :], e[:, c, :], 0.5, xh[:, c, :],
                    op0=ALU.mult, op1=ALU.mult, accum_out=num2[:, c:c + 1],
                )

            # ---- sigmoid(s) = 0.5*tanh(s/2) + 0.5; th = tanh(num2/sen) ----
            th = work.tile([128, HH], BF16, tag="th", name="th")
            for c in range(HH):
                nc.scalar.activation(
                    th[:, c:c + 1], num2[:, c:c + 1], AF.Tanh,
                    scale=recn[:, c:c + 1],
                )

            # ---- fv_{t+1} matmuls (only need xh) ----
            psf2s = []
            for c in range(HH):
                psf2 = psA.tile([128, 128], F32, tag="ps", name="psf2")
                psf2s.append(psf2)
                for k in range(HH):
                    nc.tensor.matmul(
                        psf2[:], wmat("A_uv1", k, c), xh[:, k, :],
                        start=(k == 0), stop=(k == HH - 1),
                    )

            # ---- rank-1 term vb = A_uv2h-matvec(th) + c_uv ----
            vb = work.tile([128, HH], F32, tag="vb", name="vb")
            for c in range(HH):
                psv = psA.tile([128, 1], F32, tag="psz", name="psv", bufs=2)
                for k in range(HH):
                    nc.tensor.matmul(
                        psv[:], wmat("A_uv2h", k, c), th[:, k:k + 1],
                        start=(k == 0), stop=(k == HH - 1),
                    )
                nc.vector.tensor_tensor(
                    vb[:, c:c + 1], psv[:], bias("c_uv", c), op=ALU.add
                )

            # ---- fv_{t+1} = xh @ Wuv1.T + vb ----
            fvn = state.tile([128, HH, 128], BF16, tag="fvT", name="fvn")
            for c in range(HH):
                nc.scalar.activation(
                    fvn[:, c, :], psf2s[c][:], AF.Identity, bias=vb[:, c:c + 1]
                )
            fvs.append(fvn)
            if t_step < NSTEP - 1:
                psP = psPn
                started = [True, True]

        # ---- out = fv_3 @ W_oup.T + b_oup (token-on-partition orientation),
        #      two free-halves so the first output DMA overlaps the second
        #      half's matmuls ----
        fv3 = fvs[NSTEP]
        HF = F // 2
        for h2 in range(2):
            off = h2 * HF
            pso = psO.tile([128, HF], F32, tag="pso", name="pso")
            nc.tensor.matmul(
                pso[:], ones_row[:], b_oup_sb[:, off:off + HF],
                start=True, stop=False,
            )
            for k in range(HH):
                nc.tensor.matmul(
                    pso[:], fv3[:, k, :], seg3[:, k * F + off:k * F + off + HF],
                    start=False, stop=(k == HH - 1),
                )
            out_sb = work.tile([128, HF], F32, tag="out", name="out_sb", bufs=2)
            nc.vector.tensor_copy(out_sb[:], pso[:])
            nc.sync.dma_start(io["out"][:, off:off + HF], out_sb[:])


_NC_CACHE = []


def _build():
    if _NC_CACHE:
        return _NC_CACHE[0]
    nc = bacc.Bacc("TRN2", target_bir_lowering=False, debug=False,
                   num_devices=NCORES)
    io = {}
    io["feat"] = nc.dram_tensor("feat", (L, F), F32, kind="ExternalInput").ap()
    io["mask"] = nc.dram_tensor("mask", (L,), F32, kind="ExternalInput").ap()
    for name, shape, dt in _W_NAMES:
        io[name] = nc.dram_tensor(name, shape, dt, kind="ExternalInput").ap()
    io["out"] = nc.dram_tensor("out", (L, F), F32, kind="ExternalOutput").ap()
    with tile.TileContext(nc) as tc:
        _emit(tc, io)
    nc.compile()
    _NC_CACHE.append(nc)
    return nc


def _dev_mat(w):
    """(K, M) in-first weight -> device layout (128, K/128 * M)."""
    K, M = w.shape
    return w.reshape(K // 128, 128, M).transpose(1, 0, 2).reshape(128, -1)


def _prep_weights(inputs):
    """Host-side weight precombination (float64) + device-layout packing."""
    g = {k: np.asarray(v, np.float64) for k, v in inputs.items()}
    h = H
    Wfe1T = g["W_fe"][:, :h].T           # (h, h)
    U1 = g["W_ue"][:, :h].T
    U2 = g["W_ue"][:, h:].T
    M1 = Wfe1T @ U1
    M0 = M1 + Wfe1T @ U2
    A = g["W_attn"].T
    A_uv2 = g["W_uv"][:, h:].T
    mats = {
        "A_agg": g["W_agg"].T,
        "G1": M0 @ A,
        "G2": M1 @ A,
        "G3": M0 @ U2 @ A,
        "G4": M1 @ U2 @ A,
        "G5": M0 @ U2 @ U2 @ A,
        "A_uv1": g["W_uv"][:, :h].T,
        "A_uv2h": 0.5 * A_uv2,
    }
    c_uv = g["b_uv"] + 0.5 * A_uv2.sum(axis=0)
    consts = np.concatenate(
        [g["b_inp"].reshape(HH, 128).T, g["b_agg"].reshape(HH, 128).T,
         c_uv.reshape(HH, 128).T],
        axis=1,
    )
    w = {"seg0": _dev_mat(g["W_inp"].T)}
    for segn, names in _SEG_ORDER.items():
        w[segn] = np.concatenate([_dev_mat(mats[nm]) for nm in names], axis=1)
    w["seg3"] = _dev_mat(g["W_oup"].T)
    w["b_oup_row"] = g["b_oup"][None, :]
    out = {k: np.ascontiguousarray(v.astype(np.float32)).astype(ml_dtypes.bfloat16)
           for k, v in w.items()}
    out["consts"] = np.ascontiguousarray(consts, dtype=np.float32)
    return out


def kernel(**inputs) -> np.ndarray:
    nc = _build()
    w = _prep_weights(inputs)
    feat = np.ascontiguousarray(np.asarray(inputs["feat"], np.float32))
    mask = np.ascontiguousarray(np.asarray(inputs["mask"], np.float32))
    assert feat.shape == (NCORES, L, F), feat.shape

    in_maps = []
    for c in range(NCORES):
        im = {"feat": feat[c], "mask": mask[c]}
        im.update(w)
        in_maps.append(im)

    res = bass_utils.run_bass_kernel_spmd(nc, in_maps, core_ids=list(range(NCORES)))
    out = np.stack([res.results[c]["out"] for c in range(NCORES)], axis=0)
    return out.astype(np.float32)


if __name__ == "__main__":
    rng = np.random.default_rng(0)
    demo = {
        "feat": rng.standard_normal((NCORES, L, F)).astype(np.float32),
        "mask": np.ones((NCORES, L), np.float32),
    }
    for nm, shape in [("W_inp", (H, F)), ("b_inp", (H,)), ("W_oup", (F, H)),
                      ("b_oup", (F,)), ("W_fe", (H, 2 * H)), ("b_fe", (H,)),
                      ("W_ue", (H, 2 * H)), ("b_ue", (H,)), ("W_agg", (H, H)),
                      ("b_agg", (H,)), ("W_uv", (H, 2 * H)), ("b_uv", (H,)),
                      ("W_attn", (H, H)), ("b_attn", (H,))]:
        demo[nm] = (rng.standard_normal(shape) * 0.05).astype(np.float32)
    y = kernel(**demo)
    print("kernel output:", y.shape, y.dtype)


# revision 6
# speedup vs baseline: 1.5746x; 1.1939x over previous
"""Trainium2 Bass kernel for nn_EdgeFocusedGraphNetwork.

Math: the reference's edge tensor fe[b,i,j,:] stays rank-structured for the
whole computation -- every edge update is affine and the injected new_e is an
outer sum, so fe = X[b,i,:] + Y[b,j,:] + c[:] inductively. The softmax over the
source index i is shift-invariant, which cancels the Y and c components, and
the softmax weights / aggregation become independent of j. The whole network
therefore collapses exactly (in real arithmetic) to (l, h)-sized operations per
batch element.

v3 goes one step further than expanding P_t through W_attn: the whole
recurrence is rewritten over xz_t = (fv_t @ W_agg.T) * mask and the per-h
scalars th_t = tanh(s_t/2) (sigmoid(s) = 0.5 tanh(s/2) + 0.5, and tanh shares
the exp LUT set).  Since fv_{t+1} = (xz_t + b_agg) @ Wuv1.T + sigmoid @ Wuv2.T
+ b_uv is affine in (xz_t, th_t, 1), every downstream consumer composes on the
host:

    P~_t  = fv0 @ G_{t,0} + sum_{s<t} xz_s @ GU_{t,s}   (token-constant terms
            dropped -- they shift softmax logits uniformly over i and scale
            num/den equally, so the attention output is unchanged; |P~| < 0.5
            so the max-free exp stays safe)
    z_{t+1} = xz_t @ AB + th_t (x) AVh + c_z (x) 1
    out   = xz_2 @ OU1 + th_2 (x) OU2h + c_out (x) 1

This removes fv materialization, the psf2/psv/vb chain and one full
Act->PE->DVE->Act round trip per step; the per-step critical path is just
xz (DVE) -> num via scalar_tensor_tensor accum (DVE, folds +b_agg) ->
tanh (Act, scale=0.5/den from exp's fused accum_out) -> AV/c_z matmuls (PE)
-> next xz.  The rank-1 th terms enter PSUM via stride-0 broadcast APs.

Sharding: data-parallel over batch, one graph per NeuronCore; all matmul
operands bf16 (1 PE cycle/row, half the DMA bytes), PSUM fp32.  Weight
segments are issued in need order across the SP/Act HWDGE lanes and the
gpsimd SWDGE lane (mask/consts/rows), with the two output DMAs split across
SP and Act so their issue overheads overlap.
"""

import sys

for _p in ("/opt/trn_rl_repo",):
    if _p not in sys.path:
        sys.path.insert(0, _p)

from contextlib import ExitStack

import numpy as np
import ml_dtypes

import concourse.bass as bass
import concourse.tile as tile
from concourse import bacc, mybir, bass_utils
from concourse.masks import make_identity

F32 = mybir.dt.float32
BF16 = mybir.dt.bfloat16
L = 128          # tokens per graph
H = 256          # inner width
F = 512          # in/out feature width
NSTEP = 3
NCORES = 8
HH = H // 128    # 2 feature half-blocks
FH = F // 128    # 4 feature blocks

AF = mybir.ActivationFunctionType
ALU = mybir.AluOpType
AX = mybir.AxisListType

SEGI_COLS = FH * H          # A_inp
SEGP_COLS = 2 * HH * H      # a pair of HxH matrices
SEGO_COLS = HH * F          # an HxF matrix
NCONST = 2 * HH             # b_inp | b_agg columns (fp32)

_W_NAMES = [
    ("segI", (128, SEGI_COLS), BF16),
    ("segA", (128, SEGP_COLS), BF16),   # A_agg | G1
    ("segD", (128, SEGP_COLS), BF16),   # G3 | G5
    ("segC", (128, SEGP_COLS), BF16),   # GUa | GUb
    ("segB", (128, SEGP_COLS), BF16),   # AB | AVh
    ("segO1", (128, SEGO_COLS), BF16),  # OU1
    ("segO2", (128, SEGO_COLS), BF16),  # OU2h
    ("crows", (1, H + F), BF16),        # c_z row | c_out row
    ("consts", (128, NCONST), F32),
]

_SEG_ORDER = {
    "segA": ("A_agg", "G1"),
    "segD": ("G3", "G5"),
    "segC": ("GUa", "GUb"),
    "segB": ("AB", "AVh"),
}

# P~_t = fv0 @ _G0[t] + sum_{s<t} xz_s @ _GX[t][s]
_G0 = ["G1", "G3", "G5"]
_GX = [[], ["GUa"], ["GUb", "GUa"]]


def _emit(tc, io):
    nc = tc.nc
    with ExitStack() as ctx:
        const = ctx.enter_context(tc.tile_pool(name="const", bufs=1))
        state = ctx.enter_context(tc.tile_pool(name="state", bufs=4))
        work = ctx.enter_context(tc.tile_pool(name="work", bufs=3))
        psP = ctx.enter_context(tc.tile_pool(name="psP", bufs=4, space="PSUM"))
        psZ = ctx.enter_context(tc.tile_pool(name="psZ", bufs=2, space="PSUM"))
        psO = ctx.enter_context(tc.tile_pool(name="psO", bufs=2, space="PSUM"))

        # ---- DMAs in need order across lanes: SP / Act (HWDGE), Pool (SWDGE)
        feat_sb = const.tile([128, F], F32)
        nc.sync.dma_start(feat_sb[:], io["feat"])           # SP 1
        segI = const.tile([128, SEGI_COLS], BF16)
        nc.scalar.dma_start(segI[:], io["segI"])            # Act 1

        consts = const.tile([128, NCONST], F32)
        nc.gpsimd.dma_start(consts[:], io["consts"])        # Pool 1

        segA = const.tile([128, SEGP_COLS], BF16)
        nc.sync.dma_start(segA[:], io["segA"])              # SP 2
        segD = const.tile([128, SEGP_COLS], BF16)
        nc.scalar.dma_start(segD[:], io["segD"])            # Act 2

        ident = const.tile([128, 128], F32)
        make_identity(nc, ident[:])                         # Pool compute

        maskb = const.tile([128, L], F32)  # mask broadcast to all partitions
        m = io["mask"]
        nc.gpsimd.dma_start(                                 # Pool 2
            maskb[:],
            bass.AP(tensor=m.tensor, offset=m.offset, ap=[[0, 128]] + list(m.ap)),
        )

        segC = const.tile([128, SEGP_COLS], BF16)
        nc.sync.dma_start(segC[:], io["segC"])              # SP 3
        segB = const.tile([128, SEGP_COLS], BF16)
        nc.scalar.dma_start(segB[:], io["segB"])            # Act 3

        crows = const.tile([1, H + F], BF16)
        nc.gpsimd.dma_start(crows[:], io["crows"])          # Pool 3

        segO1 = const.tile([128, SEGO_COLS], BF16)
        nc.sync.dma_start(segO1[:], io["segO1"])            # SP 4
        segO2 = const.tile([128, SEGO_COLS], BF16)
        nc.scalar.dma_start(segO2[:], io["segO2"])          # Act 4

        ones_row = const.tile([1, 128], BF16)
        nc.vector.memset(ones_row[:], 1.0)
        ones_col = const.tile([128, 1], F32)
        nc.vector.memset(ones_col[:], 1.0)

        def a_inp(k, c):
            o = k * H + c * 128
            return segI[:, o:o + 128]

        _b_off = {"b_inp": 0, "b_agg": HH}

        def bias(name, c):
            o = _b_off[name] + c
            return consts[:, o:o + 1]

        _w_seg = {}
        for segt, names in (
            (segA, _SEG_ORDER["segA"]),
            (segD, _SEG_ORDER["segD"]),
            (segC, _SEG_ORDER["segC"]),
            (segB, _SEG_ORDER["segB"]),
        ):
            for i, nm in enumerate(names):
                _w_seg[nm] = (segt, i * HH * H)

        def wmat(name, k, c):
            t, base = _w_seg[name]
            o = base + k * H + c * 128
            return t[:, o:o + 128]

        # ---- featT[p, k, l] = feat[l, 128k + p] via PE transposes ----
        featT = const.tile([128, FH, 128], BF16)
        for k in range(FH):
            pst = psO.tile([128, 256], F32, tag="pso", name="pst")
            nc.tensor.transpose(
                pst[:, :128], feat_sb[:, k * 128:(k + 1) * 128], ident[:]
            )
            nc.vector.tensor_copy(featT[:, k, :], pst[:, :128])

        # ---- fv_0 = feat @ W_inp.T + b_inp (feature-on-partition layout) ----
        fv0 = state.tile([128, HH, 128], BF16, tag="st", name="fv0")
        for c in range(HH):
            psf = psO.tile([128, 256], F32, tag="pso", name="psf")
            for k in range(FH):
                nc.tensor.matmul(
                    psf[:, :128], a_inp(k, c), featT[:, k, :],
                    start=(k == 0), stop=(k == FH - 1),
                )
            nc.scalar.activation(
                fv0[:, c, :], psf[:, :128], AF.Identity, bias=bias("b_inp", c)
            )

        # ---- lead-in: P~_0 = G1 @ fv0, z_0 = A_agg @ fv0 ----
        pp = [psP.tile([128, 128], F32, tag="pp", name="pp0") for _ in range(HH)]
        for c in range(HH):
            for k in range(HH):
                nc.tensor.matmul(
                    pp[c][:], wmat("G1", k, c), fv0[:, k, :],
                    start=(k == 0), stop=(k == HH - 1),
                )
        pz = [psZ.tile([128, 128], F32, tag="pz", name="pz0") for _ in range(HH)]
        for c in range(HH):
            for k in range(HH):
                nc.tensor.matmul(
                    pz[c][:], wmat("A_agg", k, c), fv0[:, k, :],
                    start=(k == 0), stop=(k == HH - 1),
                )

        xzs = []
        for t_step in range(NSTEP):
            last = t_step == NSTEP - 1

            # ---- e = exp(P~) with fused denominator; xz = z * mask ----
            e = work.tile([128, HH, 128], BF16, tag="e", name="e")
            sen = work.tile([128, HH], F32, tag="sen", name="sen")
            for c in range(HH):
                nc.scalar.activation(
                    e[:, c, :], pp[c][:], AF.Exp, accum_out=sen[:, c:c + 1]
                )
            xz = state.tile([128, HH, 128], BF16, tag="st", name="xz")
            for c in range(HH):
                nc.vector.tensor_tensor(xz[:, c, :], pz[c][:], maskb[:], op=ALU.mult)
            xzs.append(xz)

            # recnh = 0.5 / den (off the critical path; recn slack vs num)
            recn = work.tile([128, HH], F32, tag="recn", name="recn")
            nc.vector.reciprocal(recn[:], sen[:])
            recnh = work.tile([128, HH], F32, tag="recnh", name="recnh")
            nc.vector.scalar_tensor_tensor(
                recnh[:], recn[:], 0.5, ones_col[:].broadcast_to([128, HH]),
                op0=ALU.mult, op1=ALU.mult,
            )

            # ---- num[h] = sum_i e * (xz + b_agg), fused via accum_out ----
            prod = work.tile([128, HH, 128], BF16, tag="prod", name="prod", bufs=2)
            num = work.tile([128, HH], F32, tag="num", name="num")
            for c in range(HH):
                nc.vector.scalar_tensor_tensor(
                    prod[:, c, :], xz[:, c, :], bias("b_agg", c), e[:, c, :],
                    op0=ALU.add, op1=ALU.mult, accum_out=num[:, c:c + 1],
                )

            # ---- th = tanh(s/2) = tanh(num * recnh) ----
            th = work.tile([128, HH], BF16, tag="th", name="th")
            for c in range(HH):
                nc.scalar.activation(
                    th[:, c:c + 1], num[:, c:c + 1], AF.Tanh,
                    scale=recnh[:, c:c + 1],
                )

            # ---- P~_{t+1}: all terms available once xz_t exists ----
            if not last:
                ppn = [
                    psP.tile([128, 128], F32, tag="pp", name="ppn")
                    for _ in range(HH)
                ]
                nterm = 1 + len(_GX[t_step + 1])
                for c in range(HH):
                    i = 0
                    for k in range(HH):
                        nc.tensor.matmul(
                            ppn[c][:], wmat(_G0[t_step + 1], k, c), fv0[:, k, :],
                            start=(k == 0), stop=False,
                        )
                    for s, gn in enumerate(_GX[t_step + 1]):
                        i += 1
                        for k in range(HH):
                            nc.tensor.matmul(
                                ppn[c][:], wmat(gn, k, c), xzs[s][:, k, :],
                                start=False,
                                stop=(i == nterm - 1 and k == HH - 1),
                            )

                # ---- z_{t+1} = xz_t @ AB + th (x) AVh + c_z (x) ones ----
                pzn = [
                    psZ.tile([128, 128], F32, tag="pz", name="pzn")
                    for _ in range(HH)
                ]
                for c in range(HH):
                    for k in range(HH):
                        nc.tensor.matmul(
                            pzn[c][:], wmat("AB", k, c), xz[:, k, :],
                            start=(k == 0), stop=False,
                        )
                    for k in range(HH):
                        nc.tensor.matmul(
                            pzn[c][:], wmat("AVh", k, c),
                            th[:, k:k + 1].broadcast_to([128, 128]),
                            start=False, stop=False,
                        )
                    nc.tensor.matmul(
                        pzn[c][:], crows[:, c * 128:(c + 1) * 128], ones_row[:],
                        start=False, stop=True,
                    )
                pp = ppn
                pz = pzn

        # ---- out = xz_2 @ OU1 + th_2 (x) OU2h + c_out (x) ones ----
        xz2, th2 = xzs[-1], th
        HF = F // 2
        for h2 in range(2):
            off = h2 * HF
            psrt = psO.tile([128, 256], F32, tag="pso", name="psrt")
            psr = psrt[0:1, :HF]
            for k in range(HH):
                nc.tensor.matmul(
                    psr, th2[:, k:k + 1], segO2[:, k * F + off:k * F + off + HF],
                    start=(k == 0), stop=(k == HH - 1),
                )
            r_sb = work.tile([1, HF], BF16, tag="rsb", name="r_sb", bufs=2)
            nc.vector.tensor_tensor(
                r_sb[:], psr, crows[:, H + off:H + off + HF], op=ALU.add
            )
            psot = psO.tile([128, 256], F32, tag="pso", name="pso")
            pso = psot
            for k in range(HH):
                nc.tensor.matmul(
                    pso[:], xz2[:, k, :], segO1[:, k * F + off:k * F + off + HF],
                    start=(k == 0), stop=False,
                )
            nc.tensor.matmul(pso[:], ones_row[:], r_sb[:], start=False, stop=True)
            out_sb = work.tile([128, HF], F32, tag="out", name="out_sb", bufs=2)
            nc.vector.tensor_copy(out_sb[:], pso[:])
            eng = nc.sync if h2 == 0 else nc.scalar
            eng.dma_start(io["out"][:, off:off + HF], out_sb[:])


_NC_CACHE = []


def _build():
    if _NC_CACHE:
        return _NC_CACHE[0]
    nc = bacc.Bacc("TRN2", target_bir_lowering=False, debug=False,
                   num_devices=NCORES)
    io = {}
    io["feat"] = nc.dram_tensor("feat", (L, F), F32, kind="ExternalInput").ap()
    io["mask"] = nc.dram_tensor("mask", (L,), F32, kind="ExternalInput").ap()
    for name, shape, dt in _W_NAMES:
        io[name] = nc.dram_tensor(name, shape, dt, kind="ExternalInput").ap()
    io["out"] = nc.dram_tensor("out", (L, F), F32, kind="ExternalOutput").ap()
    with tile.TileContext(nc) as tc:
        _emit(tc, io)
    nc.compile()
    _NC_CACHE.append(nc)
    return nc


def _dev_mat(w):
    """(K, M) in-first weight -> device layout (128, K/128 * M)."""
    K, M = w.shape
    return w.reshape(K // 128, 128, M).transpose(1, 0, 2).reshape(128, -1)


def _prep_weights(inputs):
    """Host-side weight precombination (float64) + device-layout packing."""
    g = {k: np.asarray(v, np.float64) for k, v in inputs.items()}
    h = H
    Wfe1T = g["W_fe"][:, :h].T
    U1 = g["W_ue"][:, :h].T
    U2 = g["W_ue"][:, h:].T
    M1 = Wfe1T @ U1
    M0 = M1 + Wfe1T @ U2
    A = g["W_attn"].T
    A_agg = g["W_agg"].T
    A_uv1 = g["W_uv"][:, :h].T
    A_uv2 = g["W_uv"][:, h:].T
    A_oup = g["W_oup"].T
    G2 = M1 @ A
    G4 = M1 @ U2 @ A
    mats = {
        "A_agg": A_agg,
        "G1": M0 @ A,
        "G3": M0 @ U2 @ A,
        "G5": M0 @ U2 @ U2 @ A,
        "GUa": A_uv1 @ G2,
        "GUb": A_uv1 @ G4,
        "AB": A_uv1 @ A_agg,
        "AVh": 0.5 * (A_uv2 @ A_agg),
    }
    c_fv = g["b_agg"] @ A_uv1 + 0.5 * A_uv2.sum(axis=0) + g["b_uv"]
    c_z = c_fv @ A_agg
    c_out = c_fv @ A_oup + g["b_oup"]
    w = {"segI": _dev_mat(g["W_inp"].T)}
    for segn, names in _SEG_ORDER.items():
        w[segn] = np.concatenate([_dev_mat(mats[nm]) for nm in names], axis=1)
    w["segO1"] = _dev_mat(A_uv1 @ A_oup)
    w["segO2"] = _dev_mat(0.5 * (A_uv2 @ A_oup))
    w["crows"] = np.concatenate([c_z, c_out])[None, :]
    out = {k: np.ascontiguousarray(v.astype(np.float32)).astype(ml_dtypes.bfloat16)
           for k, v in w.items()}
    consts = np.concatenate(
        [g["b_inp"].reshape(HH, 128).T, g["b_agg"].reshape(HH, 128).T], axis=1
    )
    out["consts"] = np.ascontiguousarray(consts, dtype=np.float32)
    return out


def kernel(**inputs) -> np.ndarray:
    nc = _build()
    w = _prep_weights(inputs)
    feat = np.ascontiguousarray(np.asarray(inputs["feat"], np.float32))
    mask = np.ascontiguousarray(np.asarray(inputs["mask"], np.float32))
    assert feat.shape == (NCORES, L, F), feat.shape

    in_maps = []
    for c in range(NCORES):
        im = {"feat": feat[c], "mask": mask[c]}
        im.update(w)
        in_maps.append(im)

    res = bass_utils.run_bass_kernel_spmd(nc, in_maps, core_ids=list(range(NCORES)))
    out = np.stack([res.results[c]["out"] for c in range(NCORES)], axis=0)
    return out.astype(np.float32)


if __name__ == "__main__":
    rng = np.random.default_rng(0)
    demo = {
        "feat": rng.standard_normal((NCORES, L, F)).astype(np.float32),
        "mask": np.ones((NCORES, L), np.float32),
    }
    for nm, shape in [("W_inp", (H, F)), ("b_inp", (H,)), ("W_oup", (F, H)),
                      ("b_oup", (F,)), ("W_fe", (H, 2 * H)), ("b_fe", (H,)),
                      ("W_ue", (H, 2 * H)), ("b_ue", (H,)), ("W_agg", (H, H)),
                      ("b_agg", (H,)), ("W_uv", (H, 2 * H)), ("b_uv", (H,)),
                      ("W_attn", (H, H)), ("b_attn", (H,))]:
        demo[nm] = (rng.standard_normal(shape) * 0.05).astype(np.float32)
    y = kernel(**demo)
    print("kernel output:", y.shape, y.dtype)


# revision 7
# speedup vs baseline: 1.6324x; 1.0367x over previous
"""Trainium2 Bass kernel for nn_EdgeFocusedGraphNetwork.

Math: the reference's edge tensor fe[b,i,j,:] stays rank-structured for the
whole computation -- every edge update is affine and the injected new_e is an
outer sum, so fe = X[b,i,:] + Y[b,j,:] + c[:] inductively. The softmax over the
source index i is shift-invariant, which cancels the Y and c components, and
the softmax weights / aggregation become independent of j. The whole network
therefore collapses exactly (in real arithmetic) to (l, h)-sized operations per
batch element.

v3 goes one step further than expanding P_t through W_attn: the whole
recurrence is rewritten over xz_t = (fv_t @ W_agg.T) * mask and the per-h
scalars th_t = tanh(s_t/2) (sigmoid(s) = 0.5 tanh(s/2) + 0.5, and tanh shares
the exp LUT set).  Since fv_{t+1} = (xz_t + b_agg) @ Wuv1.T + sigmoid @ Wuv2.T
+ b_uv is affine in (xz_t, th_t, 1), every downstream consumer composes on the
host:

    P~_t  = fv0 @ G_{t,0} + sum_{s<t} xz_s @ GU_{t,s}   (token-constant terms
            dropped -- they shift softmax logits uniformly over i and scale
            num/den equally, so the attention output is unchanged; |P~| < 0.5
            so the max-free exp stays safe)
    z_{t+1} = xz_t @ AB + th_t (x) AVh + c_z (x) 1
    out   = xz_2 @ OU1 + th_2 (x) OU2h + c_out (x) 1

This removes fv materialization, the psf2/psv/vb chain and one full
Act->PE->DVE->Act round trip per step; the per-step critical path is just
xz (DVE) -> num via scalar_tensor_tensor accum (DVE, folds +b_agg) ->
tanh (Act, scale=0.5/den from exp's fused accum_out) -> AV/c_z matmuls (PE)
-> next xz.  The rank-1 th terms enter PSUM via stride-0 broadcast APs.

Sharding: data-parallel over batch, one graph per NeuronCore; all matmul
operands bf16 (1 PE cycle/row, half the DMA bytes), PSUM fp32.  Weight
segments are issued in need order across the SP/Act HWDGE lanes and the
gpsimd SWDGE lane (mask/consts/rows), with the two output DMAs split across
SP and Act so their issue overheads overlap.
"""

import sys

for _p in ("/opt/trn_rl_repo",):
    if _p not in sys.path:
        sys.path.insert(0, _p)

from contextlib import ExitStack

import numpy as np
import ml_dtypes

import concourse.bass as bass
import concourse.tile as tile
from concourse import bacc, mybir, bass_utils
from concourse.masks import make_identity

F32 = mybir.dt.float32
BF16 = mybir.dt.bfloat16
L = 128          # tokens per graph
H = 256          # inner width
F = 512          # in/out feature width
NSTEP = 3
NCORES = 8
HH = H // 128    # 2 feature half-blocks
FH = F // 128    # 4 feature blocks

AF = mybir.ActivationFunctionType
ALU = mybir.AluOpType
AX = mybir.AxisListType

SEGI_COLS = FH * H          # A_inp
SEGP_COLS = 2 * HH * H      # a pair of HxH matrices
SEGO_COLS = HH * F          # an HxF matrix
NCONST = 2 * HH             # b_inp | b_agg columns (fp32)

_W_NAMES = [
    ("segI", (128, SEGI_COLS), BF16),
    ("segA", (128, SEGP_COLS), BF16),   # A_agg | G1
    ("segD", (128, SEGP_COLS), BF16),   # G3 | G5
    ("segC", (128, SEGP_COLS), BF16),   # GUa | GUb
    ("segB", (128, SEGP_COLS), BF16),   # AB | AVh
    ("segO1", (128, SEGO_COLS), BF16),  # OU1
    ("segO2", (128, SEGO_COLS), BF16),  # OU2h
    ("crows", (1, H + F), BF16),        # c_z row | c_out row
    ("consts", (128, NCONST), F32),
]

_SEG_ORDER = {
    "segA": ("A_agg", "G1"),
    "segD": ("G3", "G5"),
    "segC": ("GUa", "GUb"),
    "segB": ("AB", "AVh"),
}

# P~_t = fv0 @ _G0[t] + sum_{s<t} xz_s @ _GX[t][s]
_G0 = ["G1", "G3", "G5"]
_GX = [[], ["GUa"], ["GUb", "GUa"]]


def _emit(tc, io):
    nc = tc.nc
    with ExitStack() as ctx:
        const = ctx.enter_context(tc.tile_pool(name="const", bufs=1))
        state = ctx.enter_context(tc.tile_pool(name="state", bufs=4))
        work = ctx.enter_context(tc.tile_pool(name="work", bufs=3))
        psP = ctx.enter_context(tc.tile_pool(name="psP", bufs=4, space="PSUM"))
        psZ = ctx.enter_context(tc.tile_pool(name="psZ", bufs=2, space="PSUM"))
        psO = ctx.enter_context(tc.tile_pool(name="psO", bufs=2, space="PSUM"))

        # ---- DMAs: all weight segments on the SP HWDGE lane in need order
        # (one DMA per ~650ns matches the shared HWDGE rate, and it keeps the
        # Activation SEQ free for the softmax chain); consts/mask/crows ride
        # the gpsimd SWDGE lane.
        feat_sb = const.tile([128, F], F32)
        nc.sync.dma_start(feat_sb[:], io["feat"])
        segI = const.tile([128, SEGI_COLS], BF16)
        nc.sync.dma_start(segI[:], io["segI"])

        consts = const.tile([128, NCONST], F32)
        nc.gpsimd.dma_start(consts[:], io["consts"])        # Pool 1

        segA = const.tile([128, SEGP_COLS], BF16)
        nc.sync.dma_start(segA[:], io["segA"])
        segB = const.tile([128, SEGP_COLS], BF16)
        nc.sync.dma_start(segB[:], io["segB"])

        ident = const.tile([128, 128], F32)
        make_identity(nc, ident[:])                         # Pool compute

        maskb = const.tile([128, L], F32)  # mask broadcast to all partitions
        m = io["mask"]
        nc.gpsimd.dma_start(                                 # Pool 2
            maskb[:],
            bass.AP(tensor=m.tensor, offset=m.offset, ap=[[0, 128]] + list(m.ap)),
        )

        segD = const.tile([128, SEGP_COLS], BF16)
        nc.sync.dma_start(segD[:], io["segD"])
        segC = const.tile([128, SEGP_COLS], BF16)
        nc.sync.dma_start(segC[:], io["segC"])

        crows = const.tile([1, H + F], BF16)
        nc.gpsimd.dma_start(crows[:], io["crows"])          # Pool 3

        segO1 = const.tile([128, SEGO_COLS], BF16)
        nc.sync.dma_start(segO1[:], io["segO1"])
        segO2 = const.tile([128, SEGO_COLS], BF16)
        nc.sync.dma_start(segO2[:], io["segO2"])

        ones_row = const.tile([1, 128], BF16)
        nc.vector.memset(ones_row[:], 1.0)
        ones_col = const.tile([128, 1], F32)
        nc.vector.memset(ones_col[:], 1.0)

        def a_inp(k, c):
            o = k * H + c * 128
            return segI[:, o:o + 128]

        _b_off = {"b_inp": 0, "b_agg": HH}

        def bias(name, c):
            o = _b_off[name] + c
            return consts[:, o:o + 1]

        _w_seg = {}
        for segt, names in (
            (segA, _SEG_ORDER["segA"]),
            (segD, _SEG_ORDER["segD"]),
            (segC, _SEG_ORDER["segC"]),
            (segB, _SEG_ORDER["segB"]),
        ):
            for i, nm in enumerate(names):
                _w_seg[nm] = (segt, i * HH * H)

        def wmat(name, k, c):
            t, base = _w_seg[name]
            o = base + k * H + c * 128
            return t[:, o:o + 128]

        # ---- featT[p, k, l] = feat[l, 128k + p] via PE transposes ----
        featT = const.tile([128, FH, 128], BF16)
        for k in range(FH):
            pst = psO.tile([128, 256], F32, tag="pso", name="pst")
            nc.tensor.transpose(
                pst[:, :128], feat_sb[:, k * 128:(k + 1) * 128], ident[:]
            )
            nc.vector.tensor_copy(featT[:, k, :], pst[:, :128])

        # ---- fv_0 = feat @ W_inp.T + b_inp (feature-on-partition layout) ----
        fv0 = state.tile([128, HH, 128], BF16, tag="st", name="fv0")
        for c in range(HH):
            psf = psO.tile([128, 256], F32, tag="pso", name="psf")
            for k in range(FH):
                nc.tensor.matmul(
                    psf[:, :128], a_inp(k, c), featT[:, k, :],
                    start=(k == 0), stop=(k == FH - 1),
                )
            nc.scalar.activation(
                fv0[:, c, :], psf[:, :128], AF.Identity, bias=bias("b_inp", c)
            )

        # ---- lead-in: P~_0 = G1 @ fv0, z_0 = A_agg @ fv0 ----
        pp = [psP.tile([128, 128], F32, tag="pp", name="pp0") for _ in range(HH)]
        pz = [psZ.tile([128, 128], F32, tag="pz", name="pz0") for _ in range(HH)]
        for k in range(HH):     # k-outer: k=0 matmuls only need fv0's c0 half
            for c in range(HH):
                nc.tensor.matmul(
                    pz[c][:], wmat("A_agg", k, c), fv0[:, k, :],
                    start=(k == 0), stop=(k == HH - 1),
                )
                nc.tensor.matmul(
                    pp[c][:], wmat("G1", k, c), fv0[:, k, :],
                    start=(k == 0), stop=(k == HH - 1),
                )

        xzs = []
        for t_step in range(NSTEP):
            last = t_step == NSTEP - 1

            # ---- e = exp(P~) with fused denominator; xz = z * mask ----
            e = work.tile([128, HH, 128], BF16, tag="e", name="e")
            sen = work.tile([128, HH], F32, tag="sen", name="sen")
            for c in range(HH):
                nc.scalar.activation(
                    e[:, c, :], pp[c][:], AF.Exp, accum_out=sen[:, c:c + 1]
                )
            xz = state.tile([128, HH, 128], BF16, tag="st", name="xz")
            for c in range(HH):
                nc.vector.tensor_tensor(xz[:, c, :], pz[c][:], maskb[:], op=ALU.mult)
            xzs.append(xz)

            # recnh = 0.5 / den (off the critical path; recn slack vs num)
            recn = work.tile([128, HH], F32, tag="recn", name="recn")
            nc.vector.reciprocal(recn[:], sen[:])
            recnh = work.tile([128, HH], F32, tag="recnh", name="recnh")
            nc.vector.scalar_tensor_tensor(
                recnh[:], recn[:], 0.5, ones_col[:].broadcast_to([128, HH]),
                op0=ALU.mult, op1=ALU.mult,
            )

            # ---- num[h] = sum_i e * (xz + b_agg), fused via accum_out ----
            prod = work.tile([128, HH, 128], BF16, tag="prod", name="prod", bufs=2)
            num = work.tile([128, HH], F32, tag="num", name="num")
            for c in range(HH):
                nc.vector.scalar_tensor_tensor(
                    prod[:, c, :], xz[:, c, :], bias("b_agg", c), e[:, c, :],
                    op0=ALU.add, op1=ALU.mult, accum_out=num[:, c:c + 1],
                )

            # ---- th = tanh(s/2) = tanh(num * recnh) ----
            th = work.tile([128, HH], BF16, tag="th", name="th")
            for c in range(HH):
                nc.scalar.activation(
                    th[:, c:c + 1], num[:, c:c + 1], AF.Tanh,
                    scale=recnh[:, c:c + 1],
                )

            # ---- P~_{t+1}: all terms available once xz_t exists ----
            if not last:
                ppn = [
                    psP.tile([128, 128], F32, tag="pp", name="ppn")
                    for _ in range(HH)
                ]
                nterm = 1 + len(_GX[t_step + 1])
                for c in range(HH):
                    i = 0
                    for k in range(HH):
                        nc.tensor.matmul(
                            ppn[c][:], wmat(_G0[t_step + 1], k, c), fv0[:, k, :],
                            start=(k == 0), stop=False,
                        )
                    for s, gn in enumerate(_GX[t_step + 1]):
                        i += 1
                        for k in range(HH):
                            nc.tensor.matmul(
                                ppn[c][:], wmat(gn, k, c), xzs[s][:, k, :],
                                start=False,
                                stop=(i == nterm - 1 and k == HH - 1),
                            )

                # ---- z_{t+1} = xz_t @ AB + th (x) AVh + c_z (x) ones ----
                pzn = [
                    psZ.tile([128, 128], F32, tag="pz", name="pzn")
                    for _ in range(HH)
                ]
                for c in range(HH):
                    for k in range(HH):
                        nc.tensor.matmul(
                            pzn[c][:], wmat("AB", k, c), xz[:, k, :],
                            start=(k == 0), stop=False,
                        )
                    for k in range(HH):
                        nc.tensor.matmul(
                            pzn[c][:], wmat("AVh", k, c),
                            th[:, k:k + 1].broadcast_to([128, 128]),
                            start=False, stop=False,
                        )
                    nc.tensor.matmul(
                        pzn[c][:], crows[:, c * 128:(c + 1) * 128], ones_row[:],
                        start=False, stop=True,
                    )
                pp = ppn
                pz = pzn

        # ---- out = xz_2 @ OU1 + th_2 (x) OU2h + c_out (x) ones ----
        # OU1 matmuls only need xz_2, so they open both halves' PSUM groups
        # first; the th_2-dependent rank-1 row closes each group afterwards.
        xz2, th2 = xzs[-1], th
        HF = F // 2
        psos, psrs, rsbs = [], [], []
        for h2 in range(2):
            off = h2 * HF
            pso = psO.tile([128, 256], F32, tag="pso", name="pso")
            psos.append(pso)
            for k in range(HH):
                nc.tensor.matmul(
                    pso[:], xz2[:, k, :], segO1[:, k * F + off:k * F + off + HF],
                    start=(k == 0), stop=False,
                )
        for h2 in range(2):
            off = h2 * HF
            psr = psZ.tile([1, HF], F32, tag="pz", name="psr")
            psrs.append(psr)
            for k in range(HH):
                nc.tensor.matmul(
                    psr[:], th2[:, k:k + 1], segO2[:, k * F + off:k * F + off + HF],
                    start=(k == 0), stop=(k == HH - 1),
                )
        for h2 in range(2):
            off = h2 * HF
            r_sb = work.tile([1, HF], BF16, tag="rsb", name="r_sb", bufs=2)
            rsbs.append(r_sb)
            nc.vector.tensor_tensor(
                r_sb[:], psrs[h2][:], crows[:, H + off:H + off + HF], op=ALU.add
            )
        for h2 in range(2):
            off = h2 * HF
            nc.tensor.matmul(
                psos[h2][:], ones_row[:], rsbs[h2][:], start=False, stop=True
            )
            out_sb = work.tile([128, HF], F32, tag="out", name="out_sb", bufs=2)
            nc.vector.tensor_copy(out_sb[:], psos[h2][:])
            eng = nc.sync if h2 == 0 else nc.scalar
            eng.dma_start(io["out"][:, off:off + HF], out_sb[:])


_NC_CACHE = []


def _build():
    if _NC_CACHE:
        return _NC_CACHE[0]
    nc = bacc.Bacc("TRN2", target_bir_lowering=False, debug=False,
                   num_devices=NCORES)
    io = {}
    io["feat"] = nc.dram_tensor("feat", (L, F), F32, kind="ExternalInput").ap()
    io["mask"] = nc.dram_tensor("mask", (L,), F32, kind="ExternalInput").ap()
    for name, shape, dt in _W_NAMES:
        io[name] = nc.dram_tensor(name, shape, dt, kind="ExternalInput").ap()
    io["out"] = nc.dram_tensor("out", (L, F), F32, kind="ExternalOutput").ap()
    with tile.TileContext(nc) as tc:
        _emit(tc, io)
    nc.compile()
    _NC_CACHE.append(nc)
    return nc


def _dev_mat(w):
    """(K, M) in-first weight -> device layout (128, K/128 * M)."""
    K, M = w.shape
    return w.reshape(K // 128, 128, M).transpose(1, 0, 2).reshape(128, -1)


def _prep_weights(inputs):
    """Host-side weight precombination (float64) + device-layout packing."""
    g = {k: np.asarray(v, np.float64) for k, v in inputs.items()}
    h = H
    Wfe1T = g["W_fe"][:, :h].T
    U1 = g["W_ue"][:, :h].T
    U2 = g["W_ue"][:, h:].T
    M1 = Wfe1T @ U1
    M0 = M1 + Wfe1T @ U2
    A = g["W_attn"].T
    A_agg = g["W_agg"].T
    A_uv1 = g["W_uv"][:, :h].T
    A_uv2 = g["W_uv"][:, h:].T
    A_oup = g["W_oup"].T
    G2 = M1 @ A
    G4 = M1 @ U2 @ A
    mats = {
        "A_agg": A_agg,
        "G1": M0 @ A,
        "G3": M0 @ U2 @ A,
        "G5": M0 @ U2 @ U2 @ A,
        "GUa": A_uv1 @ G2,
        "GUb": A_uv1 @ G4,
        "AB": A_uv1 @ A_agg,
        "AVh": 0.5 * (A_uv2 @ A_agg),
    }
    c_fv = g["b_agg"] @ A_uv1 + 0.5 * A_uv2.sum(axis=0) + g["b_uv"]
    c_z = c_fv @ A_agg
    c_out = c_fv @ A_oup + g["b_oup"]
    w = {"segI": _dev_mat(g["W_inp"].T)}
    for segn, names in _SEG_ORDER.items():
        w[segn] = np.concatenate([_dev_mat(mats[nm]) for nm in names], axis=1)
    w["segO1"] = _dev_mat(A_uv1 @ A_oup)
    w["segO2"] = _dev_mat(0.5 * (A_uv2 @ A_oup))
    w["crows"] = np.concatenate([c_z, c_out])[None, :]
    out = {k: np.ascontiguousarray(v.astype(np.float32)).astype(ml_dtypes.bfloat16)
           for k, v in w.items()}
    consts = np.concatenate(
        [g["b_inp"].reshape(HH, 128).T, g["b_agg"].reshape(HH, 128).T], axis=1
    )
    out["consts"] = np.ascontiguousarray(consts, dtype=np.float32)
    return out


def kernel(**inputs) -> np.ndarray:
    nc = _build()
    w = _prep_weights(inputs)
    feat = np.ascontiguousarray(np.asarray(inputs["feat"], np.float32))
    mask = np.ascontiguousarray(np.asarray(inputs["mask"], np.float32))
    assert feat.shape == (NCORES, L, F), feat.shape

    in_maps = []
    for c in range(NCORES):
        im = {"feat": feat[c], "mask": mask[c]}
        im.update(w)
        in_maps.append(im)

    res = bass_utils.run_bass_kernel_spmd(nc, in_maps, core_ids=list(range(NCORES)))
    out = np.stack([res.results[c]["out"] for c in range(NCORES)], axis=0)
    return out.astype(np.float32)


if __name__ == "__main__":
    rng = np.random.default_rng(0)
    demo = {
        "feat": rng.standard_normal((NCORES, L, F)).astype(np.float32),
        "mask": np.ones((NCORES, L), np.float32),
    }
    for nm, shape in [("W_inp", (H, F)), ("b_inp", (H,)), ("W_oup", (F, H)),
                      ("b_oup", (F,)), ("W_fe", (H, 2 * H)), ("b_fe", (H,)),
                      ("W_ue", (H, 2 * H)), ("b_ue", (H,)), ("W_agg", (H, H)),
                      ("b_agg", (H,)), ("W_uv", (H, 2 * H)), ("b_uv", (H,)),
                      ("W_attn", (H, H)), ("b_attn", (H,))]:
        demo[nm] = (rng.standard_normal(shape) * 0.05).astype(np.float32)
    y = kernel(**demo)
    print("kernel output:", y.shape, y.dtype)
